# revision 15
# baseline (speedup 1.0000x reference)
"""Trainium2 Bass kernel for nn_CausalAttnBlock (GroupNorm + per-frame spatial
self-attention + residual), SPMD over 8 NeuronCores.

Full inputs in / full outputs out. Sharding: the fused B*T frame axis (32
frames) is split 4-frames-per-core; the [C,C] projection weights are
replicated. GroupNorm(num_groups=1) statistics couple all 16 frames of a
sample, so each core computes partial (sum, sum-of-squares) over its shard and
a tiny AllReduce over each sample's 4 cores produces the global stats.

Math layout notes (per frame, C=256 channels, N=H*W=1024 positions):
  - hn = x*g' + b' with g' = gamma*rstd, b' = beta - mean*g' (per channel)
  - q = Wq hn + bq, k likewise; computed as [c_out, n] tiles (bias is
    per-partition there).
  - V^T = hn^T Wv^T computed directly as [m, c] so no transpose is needed
    later; bv is folded out: since softmax rows sum to 1, the V bias
    contributes exactly +bv to the attention output, so it is merged into
    bo' = bo + Wo bv on the host.
  - S^T = k^T q as [m(keys), n(queries)]; softmax over keys becomes a
    partition-direction sum, done with a ones-vector matmul on the PE; the
    max-subtraction is skipped (|S|/16 < ~1 for this operator's scale, exp is
    exact to ~2ulp there).
  - Z = sum_m E^T is built from a single free-axis DVE reduce over key
    subtiles plus one ones-vector matmul over the partition axis.
  - O = V E^T accumulated over key chunks (unnormalized); P_raw = Wo O is
    parked in fp32. The softmax 1/Z is a column scale, which commutes with
    the output projection, so it is applied to P_raw at the very end.
  - Batched tail (all 4 frames at once, so the ACT Ln/Exp table set is
    switched once per kernel, not per frame): R = exp(-ln Z), broadcast to
    128 partitions with a K=1 ones matmul, then y = x + P_raw*R + bo'.
All matmuls run in bf16 (inputs rounded once, fp32 PSUM accumulation).

This axon-tunneled environment has a large per-instruction dispatch cost and
~0.5 ms-class DRAM-write DMAs, so the structure above also minimizes
instruction and DMA count (one output DMA per frame, no DRAM round-trip for
the softmax normalizer, V^T PSUM packed 4-chunks-per-bank-pair).
"""

import numpy as np
import ml_dtypes

import jax
import concourse.bass as bass
import concourse.bacc as bacc
import concourse.tile as tile
from concourse import bass2jax, mybir
from jax.experimental.shard_map import shard_map
from jax.sharding import Mesh, PartitionSpec

# Problem shape (hardcoded per harness contract)
B, C, T, H, W = 2, 256, 16, 32, 32
N = H * W                 # 1024 positions per frame
F = B * T                 # 32 frames
NCORES = 8
FPC = F // NCORES         # 4 frames per core
CS = C // 128             # 2 channel subtiles
EPS = 1e-6
CNT = C * T * H * W       # elements per sample for groupnorm stats
BF16 = mybir.dt.bfloat16
F32 = mybir.dt.float32

_CACHE = {}


def build_nc(repeat: int = 1, collective: bool = True, ablate: str = '', stats: bool = True, bigdma: bool = False):
    """Build the per-core Bass program (identical on all cores)."""
    nc = bacc.Bacc("TRN2", target_bir_lowering=False, debug=False,
                   num_devices=NCORES)

    xin = nc.dram_tensor("xin", [128, CS, FPC, N], F32, kind="ExternalInput")
    wq = nc.dram_tensor("wq", [128, CS, C], BF16, kind="ExternalInput")
    wk = nc.dram_tensor("wk", [128, CS, C], BF16, kind="ExternalInput")
    wv = nc.dram_tensor("wv", [128, CS, C], BF16, kind="ExternalInput")
    wo = nc.dram_tensor("wo", [128, CS, C], BF16, kind="ExternalInput")
    bqd = nc.dram_tensor("bq", [128, CS], F32, kind="ExternalInput")
    bkd = nc.dram_tensor("bk", [128, CS], F32, kind="ExternalInput")
    bod = nc.dram_tensor("bop", [128, CS], F32, kind="ExternalInput")
    gad = nc.dram_tensor("gamma", [128, CS], F32, kind="ExternalInput")
    bed = nc.dram_tensor("beta", [128, CS], F32, kind="ExternalInput")
    y = nc.dram_tensor("y", [128, CS, FPC, N], F32, kind="ExternalOutput")

    with tile.TileContext(nc) as tc:
        with (
            tc.tile_pool(name="singles", bufs=1) as singles,
            tc.tile_pool(name="frames", bufs=2) as fr,
            tc.tile_pool(name="keep", bufs=1) as keep,
            tc.tile_pool(name="psmm", bufs=3, space="PSUM") as psmm,
            tc.tile_pool(name="psz", bufs=1, space="PSUM") as psz,
            tc.tile_pool(name="dram", bufs=2, space="DRAM") as dram,
        ):
            # ---- persistent loads ----
            xts = {}
            dmae = [nc.sync, nc.scalar]
            if bigdma:
                xbig = {}
                for s in range(CS):
                    t = singles.tile([128, FPC, N], F32, tag=f"xb_{s}")
                    xbig[s] = t
                    dmae[s % 2].dma_start(t[:], xin[:, s, :, :])
                for s in range(CS):
                    for f in range(FPC):
                        xts[(s, f)] = xbig[s][:, f]
            else:
                for s in range(CS):
                    for f in range(FPC):
                        t = singles.tile([128, N], F32, tag=f"xt_{s}_{f}")
                        xts[(s, f)] = t
                        dmae[(s * FPC + f) % 2].dma_start(t[:], xin[:, s, f, :])

            wqt = singles.tile([128, CS, C], BF16)
            wkt = singles.tile([128, CS, C], BF16)
            wvt = singles.tile([128, CS, C], BF16)
            wot = singles.tile([128, CS, C], BF16)
            for wtile, wdram in ((wqt, wq), (wkt, wk), (wvt, wv), (wot, wo)):
                nc.sync.dma_start(wtile[:], wdram[:])
            bqt = singles.tile([128, CS], F32)
            bkt = singles.tile([128, CS], F32)
            bot = singles.tile([128, CS], F32)
            gat = singles.tile([128, CS], F32)
            bet = singles.tile([128, CS], F32)
            for btile, bdram in ((bqt, bqd), (bkt, bkd), (bot, bod),
                                 (gat, gad), (bet, bed)):
                nc.sync.dma_start(btile[:], bdram[:])

            ones_f = singles.tile([128, 1], F32)
            nc.vector.memset(ones_f[:], 1.0)
            ones_b = singles.tile([128, 1], BF16)
            nc.vector.memset(ones_b[:], 1.0)
            eps_t = singles.tile([128, 1], F32)
            nc.vector.memset(eps_t[:], EPS)

            if not stats:
                gp = singles.tile([128, CS], F32)
                nc.vector.memset(gp[:], 1.0)
                bp = singles.tile([128, CS], F32)
                nc.vector.memset(bp[:], 0.0)
            else:
                # ---- groupnorm stats: per-partition mean/var over this shard ----
                nchunk = CS * FPC * (N // 512)  # 16 chunks of 512
                stats = singles.tile([128, nchunk, 6], F32)
                idx = 0
                for s in range(CS):
                    for f in range(FPC):
                        for h in range(N // 512):
                            nc.vector.bn_stats(
                                out=stats[:, idx, :],
                                in_=xts[(s, f)][:, 512 * h:512 * (h + 1)],
                            )
                            idx += 1
                mv = singles.tile([128, 2], F32)
                nc.vector.bn_aggr(out=mv[:], in_=stats[:])

                # partial sums for this shard: S_p = mean*8192, SS_p = (var+mean^2)*8192
                per_part = CS * FPC * N  # 8192 elements per partition
                s2 = singles.tile([128, 2], F32)
                nc.vector.tensor_scalar_mul(s2[:, 0:1], mv[:, 0:1], float(per_part))
                msq = singles.tile([128, 1], F32)
                nc.vector.tensor_mul(msq[:], mv[:, 0:1], mv[:, 0:1])
                nc.vector.tensor_add(msq[:], msq[:], mv[:, 1:2])
                nc.vector.tensor_scalar_mul(s2[:, 1:2], msq[:], float(per_part))

                # partition-sum via ones matmul -> [1, 2]
                pstat = psz.tile([1, 2], F32, tag="z")
                nc.tensor.matmul(pstat[:], ones_f[:], s2[:], start=True, stop=True)
                ar_sb = singles.tile([1, 2], F32)
                nc.any.tensor_copy(out=ar_sb[:], in_=pstat[:])

                # AllReduce within each sample's 4 cores
                arin = dram.tile([1, 2], F32)
                arout = dram.tile([1, 2], F32)
                nc.sync.dma_start(arin[:], ar_sb[:])
                if collective:
                    nc.gpsimd.collective_compute(
                        "AllReduce", mybir.AluOpType.add,
                        replica_groups=[[0, 1, 2, 3], [4, 5, 6, 7]],
                        ins=[arin[:].opt()], outs=[arout[:].opt()],
                    )
                else:
                    nc.sync.dma_start(arout[:], arin[:])
                # broadcast [1,2] -> [128,2] so every partition computes stats
                st_bc = singles.tile([128, 2], F32)
                nc.sync.dma_start(
                    st_bc[:],
                    bass.AP(tensor=arout[:].tensor, offset=arout[:].offset,
                            ap=[[0, 128], [1, 2]]),
                )
                mean_g = singles.tile([128, 1], F32)
                nc.vector.tensor_scalar_mul(mean_g[:], st_bc[:, 0:1], 1.0 / CNT)
                var_g = singles.tile([128, 1], F32)
                nc.vector.tensor_scalar_mul(var_g[:], st_bc[:, 1:2], 1.0 / CNT)
                mg2 = singles.tile([128, 1], F32)
                nc.vector.tensor_mul(mg2[:], mean_g[:], mean_g[:])
                nc.vector.tensor_tensor(var_g[:], var_g[:], mg2[:],
                                        mybir.AluOpType.subtract)
                # rstd = exp(-0.5*ln(var+eps))  (Ln/Exp share one ACT table set)
                lnv = singles.tile([128, 1], F32)
                nc.scalar.activation(out=lnv[:], in_=var_g[:],
                                     func=mybir.ActivationFunctionType.Ln,
                                     bias=eps_t[:], scale=1.0)
                rstd = singles.tile([128, 1], F32)
                nc.scalar.activation(out=rstd[:], in_=lnv[:],
                                     func=mybir.ActivationFunctionType.Exp,
                                     scale=-0.5)
                # g' = gamma*rstd ; b' = beta - mean*g'
                gp = singles.tile([128, CS], F32)
                nc.vector.tensor_scalar_mul(gp[:], gat[:], rstd[:])
                bp = singles.tile([128, CS], F32)
                nc.vector.tensor_scalar_mul(bp[:], gp[:], mean_g[:])
                nc.vector.tensor_tensor(bp[:], bet[:], bp[:],
                                        mybir.AluOpType.subtract)


            # ---- per-frame attention ----
            ones128 = singles.tile([1, 128], F32)
            nc.vector.memset(ones128[:], 1.0)
            zf = []
            praw = []
            for _ in range(repeat):
                zf.clear(); praw.clear()
                for f in range(FPC):
                    # normalized activations, bf16
                    hn = fr.tile([128, CS, N], BF16, tag="hn")
                    for s in range(CS):
                        nc.any.tensor_scalar(
                            out=hn[:, s, :], in0=xts[(s, f)][:],
                            scalar1=gp[:, s:s + 1], scalar2=bp[:, s:s + 1],
                            op0=mybir.AluOpType.mult, op1=mybir.AluOpType.add)

                    # V^T [m, c] = hn^T Wv^T; 4 m-chunks share one PSUM tile
                    vt = fr.tile([128, 8, C], BF16, tag="vt")
                    for g in range(2):
                        vps = psmm.tile([128, 4, C], F32, tag="mm")
                        for m4 in range(4):
                            mi = 4 * g + m4
                            for s in range(CS):
                                nc.tensor.matmul(
                                    vps[:, m4, :],
                                    hn[:, s, 128 * mi:128 * (mi + 1)],
                                    wvt[:, s, :], start=(s == 0),
                                    stop=(s == CS - 1))
                        nc.any.tensor_copy(out=vt[:, 4 * g:4 * (g + 1), :],
                                           in_=vps[:])

                    # Q, K  [c_out, n] with bias
                    qt = fr.tile([128, CS, N], BF16, tag="qt")
                    kt = fr.tile([128, CS, N], BF16, tag="kt")
                    for dst, wt, bt in ((qt, wqt, bqt), (kt, wkt, bkt)):
                        for j in range(CS):
                            pps = psmm.tile([128, N], F32, tag="mm")
                            for h in range(2):
                                hs = slice(512 * h, 512 * (h + 1))
                                for s in range(CS):
                                    nc.tensor.matmul(
                                        pps[:, hs],
                                        wt[:, s, 128 * j:128 * (j + 1)],
                                        hn[:, s, hs], start=(s == 0),
                                        stop=(s == CS - 1))
                            nc.any.tensor_scalar(
                                out=dst[:, j, :], in0=pps[:],
                                scalar1=bt[:, j:j + 1], scalar2=None,
                                op0=mybir.AluOpType.add)

                    # S^T chunks + exp -> E^T
                    et = keep.tile([128, 8, N], BF16, tag="et")
                    for mi in range(8):
                        sps = psmm.tile([128, N], F32, tag="mm")
                        for h in range(2):
                            hs = slice(512 * h, 512 * (h + 1))
                            for s in range(CS):
                                nc.tensor.matmul(
                                    sps[:, hs],
                                    kt[:, s, 128 * mi:128 * (mi + 1)],
                                    qt[:, s, hs], start=(s == 0),
                                    stop=(s == CS - 1))
                        nc.scalar.activation(
                            out=et[:, mi, :], in_=sps[:],
                            func=mybir.ActivationFunctionType.Exp,
                            scale=float(C) ** -0.5)

                    # Z[n] = sum_m E^T: free-axis partial on DVE, then a
                    # 128-partition ones-matmul closes the partition axis.
                    etr = fr.tile([128, N], F32, tag="etr")
                    nc.vector.reduce_sum(
                        out=etr[:], in_=et[:].rearrange("p j n -> p n j"),
                        axis=mybir.AxisListType.X)
                    zps = psz.tile([1, N], F32, tag="z")
                    for h in range(2):
                        hs = slice(512 * h, 512 * (h + 1))
                        nc.tensor.matmul(zps[:, hs], ones_f[:], etr[:, hs],
                                         start=True, stop=True)
                    zt = keep.tile([1, N], F32, tag=f"zf{f}")
                    nc.any.tensor_copy(out=zt[:], in_=zps[:])
                    zf.append(zt)

                    # O = V E^T (unnormalized)
                    osb = fr.tile([128, CS, N], BF16, tag="osb")
                    for j in range(CS):
                        ops = psmm.tile([128, N], F32, tag="mm")
                        for h in range(2):
                            hs = slice(512 * h, 512 * (h + 1))
                            for mi in range(8):
                                nc.tensor.matmul(
                                    ops[:, hs],
                                    vt[:, mi, 128 * j:128 * (j + 1)],
                                    et[:, mi, hs], start=(mi == 0),
                                    stop=(mi == 7))
                        nc.any.tensor_copy(out=osb[:, j, :], in_=ops[:])

                    # P_raw = Wo O, parked in fp32 until the batched tail
                    pr = keep.tile([128, CS, N], F32, tag=f"praw{f}")
                    for j in range(CS):
                        pps = psmm.tile([128, N], F32, tag="mm")
                        for h in range(2):
                            hs = slice(512 * h, 512 * (h + 1))
                            for s in range(CS):
                                nc.tensor.matmul(
                                    pps[:, hs],
                                    wot[:, s, 128 * j:128 * (j + 1)],
                                    osb[:, s, hs], start=(s == 0),
                                    stop=(s == CS - 1))
                        nc.any.tensor_copy(out=pr[:, j, :], in_=pps[:])
                    praw.append(pr)

                # ---- batched tail: R = 1/Z for all frames (one table-set
                # switch), broadcast via K=1 matmul, residual, store ----
                for f in range(FPC):
                    nc.scalar.activation(out=zf[f][:], in_=zf[f][:],
                                         func=mybir.ActivationFunctionType.Ln,
                                         scale=1.0)
                for f in range(FPC):
                    rt = keep.tile([1, N], F32, tag=f"rr{f}")
                    nc.scalar.activation(out=rt[:], in_=zf[f][:],
                                         func=mybir.ActivationFunctionType.Exp,
                                         scale=-1.0)
                    rbps = psmm.tile([128, N], F32, tag="mm")
                    for h in range(2):
                        hs = slice(512 * h, 512 * (h + 1))
                        nc.tensor.matmul(rbps[:, hs], ones128[:], rt[:, hs],
                                         start=True, stop=True)
                    pr = praw[f]
                    for j in range(CS):
                        nc.any.tensor_tensor(out=pr[:, j, :], in0=pr[:, j, :],
                                             in1=rbps[:],
                                             op=mybir.AluOpType.mult)
                        nc.any.tensor_scalar(
                            out=pr[:, j, :], in0=pr[:, j, :],
                            scalar1=bot[:, j:j + 1], scalar2=None,
                            op0=mybir.AluOpType.add)
                        nc.any.tensor_tensor(out=pr[:, j, :], in0=pr[:, j, :],
                                             in1=xts[(j, f)][:],
                                             op=mybir.AluOpType.add)
                    dmae[f % 2].dma_start(y[:, :, f, :], pr[:, :, :])

    nc.compile()
    return nc


class Runner:
    """Jitted SPMD executable for one built Bass program, reused across calls
    so the NEFF is loaded onto the devices only once."""

    def __init__(self, nc):
        bass2jax.install_neuronx_cc_hook()
        self.nc = nc
        pname = nc.partition_id_tensor.name if nc.partition_id_tensor else None
        in_names, out_names, out_avals = [], [], []
        for alloc in nc.m.functions[0].allocations:
            if not isinstance(alloc, mybir.MemoryLocationSet):
                continue
            name = alloc.memorylocations[0].name
            if alloc.kind == "ExternalInput":
                if name != pname:
                    in_names.append(name)
            elif alloc.kind == "ExternalOutput":
                out_names.append(name)
                out_avals.append(jax.core.ShapedArray(
                    tuple(alloc.tensor_shape), mybir.dt.np(alloc.dtype)))
        self.in_names, self.out_names, self.out_avals = \
            in_names, out_names, out_avals
        n_params = len(in_names)
        bind_names = in_names + out_names + ([pname] if pname else [])
        donate = tuple(range(n_params, n_params + len(out_names)))

        def _body(*args):
            operands = list(args)
            if pname:
                operands.append(bass2jax.partition_id_tensor())
            outs = bass2jax._bass_exec_p.bind(
                *operands, out_avals=tuple(out_avals),
                in_names=tuple(bind_names), out_names=tuple(out_names),
                lowering_input_output_aliases=(),
                sim_require_finite=True, sim_require_nnan=True, nc=nc)
            return tuple(outs)

        self.devices = jax.devices()[:NCORES]
        self.mesh = Mesh(np.asarray(self.devices), ("core",))
        nio = n_params + len(out_names)
        self.sharded = jax.jit(
            shard_map(_body, mesh=self.mesh,
                      in_specs=(PartitionSpec("core"),) * nio,
                      out_specs=(PartitionSpec("core"),) * len(out_names),
                      check_rep=False),
            donate_argnums=donate, keep_unused=True)

    def concat_inputs(self, in_maps):
        return [np.concatenate([np.asarray(m[n]) for m in in_maps], axis=0)
                for n in self.in_names]

    def fresh_zeros(self):
        return [np.zeros((NCORES * a.shape[0], *a.shape[1:]), a.dtype)
                for a in self.out_avals]

    def __call__(self, concat_in, zeros):
        out = self.sharded(*concat_in, *zeros)
        jax.block_until_ready(out)
        return out

    def run(self, in_maps):
        out = self(self.concat_inputs(in_maps), self.fresh_zeros())
        return [
            {n: np.asarray(out[i]).reshape(NCORES, *self.out_avals[i].shape)[c]
             for i, n in enumerate(self.out_names)}
            for c in range(NCORES)
        ]


def _get_runner(repeat: int = 1, ablate: str = ""):
    key = (repeat, ablate)
    if key not in _CACHE:
        _CACHE[key] = Runner(build_nc(repeat, ablate=ablate))
    return _CACHE[key]


def _prep_inputs(x, gamma, beta, wq, bq, wk, bk, wv, bv, wo, bo):
    """Host-side sharding / layout prep -> per-core input maps."""
    bf = ml_dtypes.bfloat16

    def wprep(w):
        # lhsT layout [ci, c_out] striped to [p, cs, c_out]
        return np.ascontiguousarray(
            w.T.reshape(CS, 128, C).transpose(1, 0, 2)).astype(bf)

    def vprep(v):
        # per-channel [C] -> [128, CS]
        return np.ascontiguousarray(v.reshape(CS, 128).T).astype(np.float32)

    wq_h, wk_h, wv_h, wo_h = wprep(wq), wprep(wk), wprep(wv), wprep(wo)
    bop = (wo.astype(np.float64) @ bv.astype(np.float64)).astype(np.float32) + bo
    shared = {
        "wq": wq_h, "wk": wk_h, "wv": wv_h, "wo": wo_h,
        "bq": vprep(bq), "bk": vprep(bk), "bop": vprep(bop),
        "gamma": vprep(gamma), "beta": vprep(beta),
    }

    frames = np.ascontiguousarray(
        x.transpose(0, 2, 1, 3, 4).reshape(F, C, N))  # [32, 256, 1024]
    in_maps = []
    for c in range(NCORES):
        sh = frames[FPC * c:FPC * (c + 1)]           # [4, 256, 1024]
        arr = np.ascontiguousarray(
            sh.transpose(1, 0, 2).reshape(CS, 128, FPC, N).transpose(1, 0, 2, 3))
        in_maps.append({"xin": arr.astype(np.float32), **shared})
    return in_maps


def _assemble(results):
    frames = np.empty((F, C, N), np.float32)
    for c in range(NCORES):
        arr = results[c]["y"]                        # [128, CS, FPC, N]
        frames[FPC * c:FPC * (c + 1)] = (
            arr.transpose(1, 0, 2, 3).reshape(C, FPC, N).transpose(1, 0, 2))
    return frames.reshape(B, T, C, H, W).transpose(0, 2, 1, 3, 4)


def kernel(**inputs):
    inputs = {k: np.asarray(v) for k, v in inputs.items()}
    in_maps = _prep_inputs(**inputs)
    runner = _get_runner()
    return _assemble(runner.run(in_maps))



# revision 17
# speedup vs baseline: 79.9817x; 79.9817x over previous
"""Trainium2 Bass kernel for nn_CausalAttnBlock (GroupNorm + per-frame spatial
self-attention + residual), SPMD over 8 NeuronCores.

Full inputs in / full outputs out. Sharding: the fused B*T frame axis (32
frames) is split 4-frames-per-core; the [C,C] projection weights are
replicated. GroupNorm(num_groups=1) statistics couple all 16 frames of a
sample, so each core computes partial (sum, sum-of-squares) over its shard and
a tiny AllReduce over each sample's 4 cores produces the global stats.

Math layout notes (per frame, C=256 channels, N=H*W=1024 positions):
  - hn = x*g' + b' with g' = gamma*rstd, b' = beta - mean*g' (per channel)
  - q = Wq hn + bq, k likewise; computed as [c_out, n] tiles (bias is
    per-partition there).
  - V^T = hn^T Wv^T computed directly as [m, c] so no transpose is needed
    later; bv is folded out: since softmax rows sum to 1, the V bias
    contributes exactly +bv to the attention output, so it is merged into
    bo' = bo + Wo bv on the host.
  - S^T = k^T q as [m(keys), n(queries)]; softmax over keys becomes a
    partition-direction sum, done with a ones-vector matmul on the PE; the
    max-subtraction is skipped (|S|/16 < ~1 for this operator's scale, exp is
    exact to ~2ulp there).
  - Z = sum_m E^T is built from a single free-axis DVE reduce over key
    subtiles plus one ones-vector matmul over the partition axis.
  - O = V E^T accumulated over key chunks (unnormalized); P_raw = Wo O is
    parked in fp32. The softmax 1/Z is a column scale, which commutes with
    the output projection, so it is applied to P_raw at the very end.
  - Batched tail (all 4 frames at once, so the ACT Ln/Exp table set is
    switched once per kernel, not per frame): R = exp(-ln Z), broadcast to
    128 partitions with a K=1 ones matmul, then y = x + P_raw*R + bo'.
All matmuls run in bf16 (inputs rounded once, fp32 PSUM accumulation).

This axon-tunneled environment has a large per-instruction dispatch cost and
~0.5 ms-class DRAM-write DMAs, so the structure above also minimizes
instruction and DMA count (one output DMA per frame, no DRAM round-trip for
the softmax normalizer, V^T PSUM packed 4-chunks-per-bank-pair).
"""

import numpy as np
import ml_dtypes

import jax
import concourse.bass as bass
import concourse.bacc as bacc
import concourse.tile as tile
from concourse import bass2jax, mybir
from jax.experimental.shard_map import shard_map
from jax.sharding import Mesh, PartitionSpec
# Problem shape (hardcoded per harness contract)
B, C, T, H, W = 2, 256, 16, 32, 32
N = H * W                 # 1024 positions per frame
F = B * T                 # 32 frames
NCORES = 8
FPC = F // NCORES         # 4 frames per core
CS = C // 128             # 2 channel subtiles
EPS = 1e-6
CNT = C * T * H * W       # elements per sample for groupnorm stats
BF16 = mybir.dt.bfloat16
F32 = mybir.dt.float32

_CACHE = {}


def build_nc(repeat: int = 1, collective: bool = True, ablate: str = '', stats: bool = True, bigdma: bool = False):
    """Build the per-core Bass program (identical on all cores)."""
    nc = bacc.Bacc("TRN2", target_bir_lowering=False, debug=False,
                   num_devices=NCORES)

    xin = nc.dram_tensor("xin", [128, CS, FPC, N], F32, kind="ExternalInput")
    wq = nc.dram_tensor("wq", [128, CS, C], BF16, kind="ExternalInput")
    wk = nc.dram_tensor("wk", [128, CS, C], BF16, kind="ExternalInput")
    wv = nc.dram_tensor("wv", [128, CS, C], BF16, kind="ExternalInput")
    wo = nc.dram_tensor("wo", [128, CS, C], BF16, kind="ExternalInput")
    bqd = nc.dram_tensor("bq", [128, CS], F32, kind="ExternalInput")
    bkd = nc.dram_tensor("bk", [128, CS], F32, kind="ExternalInput")
    bod = nc.dram_tensor("bop", [128, CS], F32, kind="ExternalInput")
    gad = nc.dram_tensor("gamma", [128, CS], F32, kind="ExternalInput")
    bed = nc.dram_tensor("beta", [128, CS], F32, kind="ExternalInput")
    y = nc.dram_tensor("y", [128, CS, FPC, N], F32, kind="ExternalOutput")

    with tile.TileContext(nc) as tc:
        with (
            tc.tile_pool(name="singles", bufs=1) as singles,
            tc.tile_pool(name="frames", bufs=2) as fr,
            tc.tile_pool(name="keep", bufs=1) as keep,
            tc.tile_pool(name="psmm", bufs=3, space="PSUM") as psmm,
            tc.tile_pool(name="psz", bufs=1, space="PSUM") as psz,
            tc.tile_pool(name="dram", bufs=2, space="DRAM") as dram,
        ):
            # ---- persistent loads ----
            xts = {}
            dmae = [nc.sync, nc.scalar]
            if bigdma:
                xbig = {}
                for s in range(CS):
                    t = singles.tile([128, FPC, N], F32, tag=f"xb_{s}")
                    xbig[s] = t
                    dmae[s % 2].dma_start(t[:], xin[:, s, :, :])
                for s in range(CS):
                    for f in range(FPC):
                        xts[(s, f)] = xbig[s][:, f]
            else:
                for s in range(CS):
                    for f in range(FPC):
                        t = singles.tile([128, N], F32, tag=f"xt_{s}_{f}")
                        xts[(s, f)] = t
                        dmae[(s * FPC + f) % 2].dma_start(t[:], xin[:, s, f, :])

            wqt = singles.tile([128, CS, C], BF16)
            wkt = singles.tile([128, CS, C], BF16)
            wvt = singles.tile([128, CS, C], BF16)
            wot = singles.tile([128, CS, C], BF16)
            for wtile, wdram in ((wqt, wq), (wkt, wk), (wvt, wv), (wot, wo)):
                nc.sync.dma_start(wtile[:], wdram[:])
            bqt = singles.tile([128, CS], F32)
            bkt = singles.tile([128, CS], F32)
            bot = singles.tile([128, CS], F32)
            gat = singles.tile([128, CS], F32)
            bet = singles.tile([128, CS], F32)
            for btile, bdram in ((bqt, bqd), (bkt, bkd), (bot, bod),
                                 (gat, gad), (bet, bed)):
                nc.sync.dma_start(btile[:], bdram[:])

            ones_f = singles.tile([128, 1], F32)
            nc.vector.memset(ones_f[:], 1.0)
            ones_b = singles.tile([128, 1], BF16)
            nc.vector.memset(ones_b[:], 1.0)
            eps_t = singles.tile([128, 1], F32)
            nc.vector.memset(eps_t[:], EPS)

            if not stats:
                gp = singles.tile([128, CS], F32)
                nc.vector.memset(gp[:], 1.0)
                bp = singles.tile([128, CS], F32)
                nc.vector.memset(bp[:], 0.0)
            else:
                # ---- groupnorm stats: per-partition mean/var over this shard ----
                nchunk = CS * FPC * (N // 512)  # 16 chunks of 512
                stats = singles.tile([128, nchunk, 6], F32)
                idx = 0
                for s in range(CS):
                    for f in range(FPC):
                        for h in range(N // 512):
                            nc.vector.bn_stats(
                                out=stats[:, idx, :],
                                in_=xts[(s, f)][:, 512 * h:512 * (h + 1)],
                            )
                            idx += 1
                mv = singles.tile([128, 2], F32)
                nc.vector.bn_aggr(out=mv[:], in_=stats[:])

                # partial sums for this shard: S_p = mean*8192, SS_p = (var+mean^2)*8192
                per_part = CS * FPC * N  # 8192 elements per partition
                s2 = singles.tile([128, 2], F32)
                nc.vector.tensor_scalar_mul(s2[:, 0:1], mv[:, 0:1], float(per_part))
                msq = singles.tile([128, 1], F32)
                nc.vector.tensor_mul(msq[:], mv[:, 0:1], mv[:, 0:1])
                nc.vector.tensor_add(msq[:], msq[:], mv[:, 1:2])
                nc.vector.tensor_scalar_mul(s2[:, 1:2], msq[:], float(per_part))

                # partition-sum via ones matmul -> [1, 2]
                pstat = psz.tile([1, 2], F32, tag="z")
                nc.tensor.matmul(pstat[:], ones_f[:], s2[:], start=True, stop=True)
                ar_sb = singles.tile([1, 2], F32)
                nc.any.tensor_copy(out=ar_sb[:], in_=pstat[:])

                # AllReduce within each sample's 4 cores
                arin = dram.tile([1, 2], F32)
                arout = dram.tile([1, 2], F32)
                nc.sync.dma_start(arin[:], ar_sb[:])
                if collective:
                    nc.gpsimd.collective_compute(
                        "AllReduce", mybir.AluOpType.add,
                        replica_groups=[[0, 1, 2, 3], [4, 5, 6, 7]],
                        ins=[arin[:].opt()], outs=[arout[:].opt()],
                    )
                else:
                    nc.sync.dma_start(arout[:], arin[:])
                # broadcast [1,2] -> [128,2] so every partition computes stats
                st_bc = singles.tile([128, 2], F32)
                nc.sync.dma_start(
                    st_bc[:],
                    bass.AP(tensor=arout[:].tensor, offset=arout[:].offset,
                            ap=[[0, 128], [1, 2]]),
                )
                mean_g = singles.tile([128, 1], F32)
                nc.vector.tensor_scalar_mul(mean_g[:], st_bc[:, 0:1], 1.0 / CNT)
                var_g = singles.tile([128, 1], F32)
                nc.vector.tensor_scalar_mul(var_g[:], st_bc[:, 1:2], 1.0 / CNT)
                mg2 = singles.tile([128, 1], F32)
                nc.vector.tensor_mul(mg2[:], mean_g[:], mean_g[:])
                nc.vector.tensor_tensor(var_g[:], var_g[:], mg2[:],
                                        mybir.AluOpType.subtract)
                # rstd = exp(-0.5*ln(var+eps))  (Ln/Exp share one ACT table set)
                lnv = singles.tile([128, 1], F32)
                nc.scalar.activation(out=lnv[:], in_=var_g[:],
                                     func=mybir.ActivationFunctionType.Ln,
                                     bias=eps_t[:], scale=1.0)
                rstd = singles.tile([128, 1], F32)
                nc.scalar.activation(out=rstd[:], in_=lnv[:],
                                     func=mybir.ActivationFunctionType.Exp,
                                     scale=-0.5)
                # g' = gamma*rstd ; b' = beta - mean*g'
                gp = singles.tile([128, CS], F32)
                nc.vector.tensor_scalar_mul(gp[:], gat[:], rstd[:])
                bp = singles.tile([128, CS], F32)
                nc.vector.tensor_scalar_mul(bp[:], gp[:], mean_g[:])
                nc.vector.tensor_tensor(bp[:], bet[:], bp[:],
                                        mybir.AluOpType.subtract)


            # ---- per-frame attention ----
            ones128 = singles.tile([1, 128], F32)
            nc.vector.memset(ones128[:], 1.0)
            zf = []
            praw = []
            for _ in range(repeat):
                zf.clear(); praw.clear()
                for f in range(FPC):
                    # normalized activations, bf16
                    hn = fr.tile([128, CS, N], BF16, tag="hn")
                    for s in range(CS):
                        nc.any.tensor_scalar(
                            out=hn[:, s, :], in0=xts[(s, f)][:],
                            scalar1=gp[:, s:s + 1], scalar2=bp[:, s:s + 1],
                            op0=mybir.AluOpType.mult, op1=mybir.AluOpType.add)

                    # V^T [m, c] = hn^T Wv^T; 4 m-chunks share one PSUM tile
                    vt = fr.tile([128, 8, C], BF16, tag="vt")
                    for g in range(2):
                        vps = psmm.tile([128, 4, C], F32, tag="mm")
                        for m4 in range(4):
                            mi = 4 * g + m4
                            for s in range(CS):
                                nc.tensor.matmul(
                                    vps[:, m4, :],
                                    hn[:, s, 128 * mi:128 * (mi + 1)],
                                    wvt[:, s, :], start=(s == 0),
                                    stop=(s == CS - 1))
                        nc.any.tensor_copy(out=vt[:, 4 * g:4 * (g + 1), :],
                                           in_=vps[:])

                    # Q, K  [c_out, n] with bias
                    qt = fr.tile([128, CS, N], BF16, tag="qt")
                    kt = fr.tile([128, CS, N], BF16, tag="kt")
                    for dst, wt, bt in ((qt, wqt, bqt), (kt, wkt, bkt)):
                        for j in range(CS):
                            pps = psmm.tile([128, N], F32, tag="mm")
                            for h in range(2):
                                hs = slice(512 * h, 512 * (h + 1))
                                for s in range(CS):
                                    nc.tensor.matmul(
                                        pps[:, hs],
                                        wt[:, s, 128 * j:128 * (j + 1)],
                                        hn[:, s, hs], start=(s == 0),
                                        stop=(s == CS - 1))
                            nc.any.tensor_scalar(
                                out=dst[:, j, :], in0=pps[:],
                                scalar1=bt[:, j:j + 1], scalar2=None,
                                op0=mybir.AluOpType.add)

                    # S^T chunks + exp -> E^T
                    et = keep.tile([128, 8, N], BF16, tag="et")
                    for mi in range(8):
                        sps = psmm.tile([128, N], F32, tag="mm")
                        for h in range(2):
                            hs = slice(512 * h, 512 * (h + 1))
                            for s in range(CS):
                                nc.tensor.matmul(
                                    sps[:, hs],
                                    kt[:, s, 128 * mi:128 * (mi + 1)],
                                    qt[:, s, hs], start=(s == 0),
                                    stop=(s == CS - 1))
                        nc.scalar.activation(
                            out=et[:, mi, :], in_=sps[:],
                            func=mybir.ActivationFunctionType.Exp,
                            scale=float(C) ** -0.5)

                    # Z[n] = sum_m E^T: free-axis partial on DVE, then a
                    # 128-partition ones-matmul closes the partition axis.
                    etr = fr.tile([128, N], F32, tag="etr")
                    nc.vector.reduce_sum(
                        out=etr[:], in_=et[:].rearrange("p j n -> p n j"),
                        axis=mybir.AxisListType.X)
                    zps = psz.tile([1, N], F32, tag="z")
                    for h in range(2):
                        hs = slice(512 * h, 512 * (h + 1))
                        nc.tensor.matmul(zps[:, hs], ones_f[:], etr[:, hs],
                                         start=True, stop=True)
                    zt = keep.tile([1, N], F32, tag=f"zf{f}")
                    nc.any.tensor_copy(out=zt[:], in_=zps[:])
                    zf.append(zt)

                    # O = V E^T (unnormalized)
                    osb = fr.tile([128, CS, N], BF16, tag="osb")
                    for j in range(CS):
                        ops = psmm.tile([128, N], F32, tag="mm")
                        for h in range(2):
                            hs = slice(512 * h, 512 * (h + 1))
                            for mi in range(8):
                                nc.tensor.matmul(
                                    ops[:, hs],
                                    vt[:, mi, 128 * j:128 * (j + 1)],
                                    et[:, mi, hs], start=(mi == 0),
                                    stop=(mi == 7))
                        nc.any.tensor_copy(out=osb[:, j, :], in_=ops[:])

                    # P_raw = Wo O, parked in fp32 until the batched tail
                    pr = keep.tile([128, CS, N], F32, tag=f"praw{f}")
                    for j in range(CS):
                        pps = psmm.tile([128, N], F32, tag="mm")
                        for h in range(2):
                            hs = slice(512 * h, 512 * (h + 1))
                            for s in range(CS):
                                nc.tensor.matmul(
                                    pps[:, hs],
                                    wot[:, s, 128 * j:128 * (j + 1)],
                                    osb[:, s, hs], start=(s == 0),
                                    stop=(s == CS - 1))
                        nc.any.tensor_copy(out=pr[:, j, :], in_=pps[:])
                    praw.append(pr)

                # ---- batched tail: R = 1/Z for all frames (one table-set
                # switch), broadcast via K=1 matmul, residual, store ----
                for f in range(FPC):
                    nc.scalar.activation(out=zf[f][:], in_=zf[f][:],
                                         func=mybir.ActivationFunctionType.Ln,
                                         scale=1.0)
                for f in range(FPC):
                    rt = keep.tile([1, N], F32, tag=f"rr{f}")
                    nc.scalar.activation(out=rt[:], in_=zf[f][:],
                                         func=mybir.ActivationFunctionType.Exp,
                                         scale=-1.0)
                    rbps = psmm.tile([128, N], F32, tag="mm")
                    for h in range(2):
                        hs = slice(512 * h, 512 * (h + 1))
                        nc.tensor.matmul(rbps[:, hs], ones128[:], rt[:, hs],
                                         start=True, stop=True)
                    pr = praw[f]
                    for j in range(CS):
                        nc.any.tensor_tensor(out=pr[:, j, :], in0=pr[:, j, :],
                                             in1=rbps[:],
                                             op=mybir.AluOpType.mult)
                        nc.any.tensor_scalar(
                            out=pr[:, j, :], in0=pr[:, j, :],
                            scalar1=bot[:, j:j + 1], scalar2=None,
                            op0=mybir.AluOpType.add)
                        nc.any.tensor_tensor(out=pr[:, j, :], in0=pr[:, j, :],
                                             in1=xts[(j, f)][:],
                                             op=mybir.AluOpType.add)
                    dmae[f % 2].dma_start(y[:, :, f, :], pr[:, :, :])

    nc.compile()
    return nc


class Runner:
    """Jitted SPMD executable for one built Bass program, reused across calls
    so the NEFF is loaded onto the devices only once."""

    def __init__(self, nc):
        bass2jax.install_neuronx_cc_hook()
        self.nc = nc
        pname = nc.partition_id_tensor.name if nc.partition_id_tensor else None
        in_names, out_names, out_avals = [], [], []
        for alloc in nc.m.functions[0].allocations:
            if not isinstance(alloc, mybir.MemoryLocationSet):
                continue
            name = alloc.memorylocations[0].name
            if alloc.kind == "ExternalInput":
                if name != pname:
                    in_names.append(name)
            elif alloc.kind == "ExternalOutput":
                out_names.append(name)
                out_avals.append(jax.core.ShapedArray(
                    tuple(alloc.tensor_shape), mybir.dt.np(alloc.dtype)))
        self.in_names, self.out_names, self.out_avals = \
            in_names, out_names, out_avals
        n_params = len(in_names)
        bind_names = in_names + out_names + ([pname] if pname else [])
        donate = tuple(range(n_params, n_params + len(out_names)))

        def _body(*args):
            operands = list(args)
            if pname:
                operands.append(bass2jax.partition_id_tensor())
            outs = bass2jax._bass_exec_p.bind(
                *operands, out_avals=tuple(out_avals),
                in_names=tuple(bind_names), out_names=tuple(out_names),
                lowering_input_output_aliases=(),
                sim_require_finite=True, sim_require_nnan=True, nc=nc)
            return tuple(outs)

        self.devices = jax.devices()[:NCORES]
        self.mesh = Mesh(np.asarray(self.devices), ("core",))
        nio = n_params + len(out_names)
        self.sharded = jax.jit(
            shard_map(_body, mesh=self.mesh,
                      in_specs=(PartitionSpec("core"),) * nio,
                      out_specs=(PartitionSpec("core"),) * len(out_names),
                      check_rep=False),
            donate_argnums=donate, keep_unused=True)

    def concat_inputs(self, in_maps):
        return [np.concatenate([np.asarray(m[n]) for m in in_maps], axis=0)
                for n in self.in_names]

    def fresh_zeros(self):
        return [np.zeros((NCORES * a.shape[0], *a.shape[1:]), a.dtype)
                for a in self.out_avals]

    def __call__(self, concat_in, zeros):
        out = self.sharded(*concat_in, *zeros)
        jax.block_until_ready(out)
        return out

    def run(self, in_maps):
        out = self(self.concat_inputs(in_maps), self.fresh_zeros())
        return [
            {n: np.asarray(out[i]).reshape(NCORES, *self.out_avals[i].shape)[c]
             for i, n in enumerate(self.out_names)}
            for c in range(NCORES)
        ]


def _get_runner(repeat: int = 1, ablate: str = ""):
    key = (repeat, ablate)
    if key not in _CACHE:
        _CACHE[key] = Runner(build_nc(repeat, ablate=ablate))
    return _CACHE[key]


def _prep_inputs(x, gamma, beta, wq, bq, wk, bk, wv, bv, wo, bo):
    """Host-side sharding / layout prep -> per-core input maps."""
    bf = ml_dtypes.bfloat16

    def wprep(w):
        # lhsT layout [ci, c_out] striped to [p, cs, c_out]
        return np.ascontiguousarray(
            w.T.reshape(CS, 128, C).transpose(1, 0, 2)).astype(bf)

    def vprep(v):
        # per-channel [C] -> [128, CS]
        return np.ascontiguousarray(v.reshape(CS, 128).T).astype(np.float32)

    wq_h, wk_h, wv_h, wo_h = wprep(wq), wprep(wk), wprep(wv), wprep(wo)
    bop = (wo.astype(np.float64) @ bv.astype(np.float64)).astype(np.float32) + bo
    shared = {
        "wq": wq_h, "wk": wk_h, "wv": wv_h, "wo": wo_h,
        "bq": vprep(bq), "bk": vprep(bk), "bop": vprep(bop),
        "gamma": vprep(gamma), "beta": vprep(beta),
    }

    frames = np.ascontiguousarray(
        x.transpose(0, 2, 1, 3, 4).reshape(F, C, N))  # [32, 256, 1024]
    in_maps = []
    for c in range(NCORES):
        sh = frames[FPC * c:FPC * (c + 1)]           # [4, 256, 1024]
        arr = np.ascontiguousarray(
            sh.transpose(1, 0, 2).reshape(CS, 128, FPC, N).transpose(1, 0, 2, 3))
        in_maps.append({"xin": arr.astype(np.float32), **shared})
    return in_maps


def _assemble(results):
    frames = np.empty((F, C, N), np.float32)
    for c in range(NCORES):
        arr = results[c]["y"]                        # [128, CS, FPC, N]
        frames[FPC * c:FPC * (c + 1)] = (
            arr.transpose(1, 0, 2, 3).reshape(C, FPC, N).transpose(1, 0, 2))
    return frames.reshape(B, T, C, H, W).transpose(0, 2, 1, 3, 4)


def kernel(**inputs):
    inputs = {k: np.asarray(v) for k, v in inputs.items()}
    in_maps = _prep_inputs(**inputs)
    runner = _get_runner()
    return _assemble(runner.run(in_maps))

